# revision 1
# baseline (speedup 1.0000x reference)
"""MetaSR super-resolution Trainium2 kernel.

Structure exploited: out_h=out_w=256 with H=W=64 LR grid means the scale
factor is exactly 4, so the nearest-neighbor gather index is iy=oy//4,
ix=ox//4 and the per-query MLP input collapses to 16 distinct subpixel
phases [dy/4, dx/4, 0.25].  The whole model becomes:

  1. h    = relu(mlp_in @ w1 + b1)              [16, 256]
  2. predw = h @ w2 + b2                        [16, 576, 3]
  3. rgb[o, 4*iy+dy, 4*ix+dx] =
       sum_{c,ki,kj} feat[c, iy+ki-1, ix+kj-1] * predw[(dy,dx), c*9+ki*3+kj, o]
     i.e. a 3x3 conv with 64 in / 48 out channels + pixel shuffle.

Sharding: data-parallel over LR rows (8 rows per core, 10-row halo band),
weights replicated; steps 1+2 are recomputed on every core (tiny).

The conv contraction (K = 9 taps x 64 ch = 576) is chunked K=128 by pairing
taps.  Each core holds the zero-padded band twice in a 128-partition tile at
free-dim offsets that differ by the two taps' shift delta, so one K=128
matmul consumes two taps without materializing the unfolded tensor:
  band free index = r*66 + x  (66-wide zero-padded rows), tap (ki,kj) shift
  = ki*66+kj; taps are paired with shift deltas 1 or 64.

Inputs are packed host-side into a few large per-core DRAM blobs, ordered by
when the kernel needs them (small weights -> first w2 chunk -> band -> rest)
so compute starts as soon as the first blob lands.  A run of dummy matmuls
(zero scratch data, overwritten by the first real conv accumulation) warms
the PE HAM clock gate during the DMA phase.

float32r mode (METASR_F32R=1): the conv matmuls run in fp32r (full-rate fp32
on the PE); band data is pre-rounded host-side and W is written as fp32r.
"""

import os

import numpy as np

try:
    import concourse.bass as bass
except ImportError:  # fall back to the repo checkout
    import sys
    sys.path.insert(0, "/opt/trn_rl_repo")
    import concourse.bass as bass
import concourse.mybir as mybir
import concourse.tile as tile
from concourse import bacc
from concourse.bass_utils import run_bass_kernel_spmd

F32 = mybir.dt.float32
F32R = mybir.dt.float32r
BF16 = mybir.dt.bfloat16
N_CORES = 8
ROWS_PER_CORE = 8          # LR rows per core
BAND_ROWS = ROWS_PER_CORE + 2
NPOS = ROWS_PER_CORE * 64  # 512 LR positions per core

# Tap order for K-chunking.  Taps t = ki*3+kj have band shift ki*66+kj:
#   t:      0   1   2   3    4    5    6    7    8
#   shift:  0   1   2   66   67   68   132  133  134
# chunk0: [t0; t1] band1 off 1 | chunk1: [t3; t2] band2 off 66
# chunk2: [t4; t5] band1 off 68 | chunk3: [t6; t7] band1 off 133
# chunk4: [t8] band2 off 134 (K=64)
TAP_ORDER = [0, 1, 3, 2, 4, 5, 6, 7, 8]
CHUNK_SPECS = [  # (band_tile_idx, rhs_offset, K)
    (0, 1, 128),
    (1, 66, 128),
    (0, 68, 128),
    (0, 133, 128),
    (1, 134, 64),
]

# blob_sm0 layout: small constants + w2 m=0 block
OFF_W1 = 0          # [3, 256]   (partitions 0-2)
OFF_MLP = 256       # [3, 16]
OFF_B1B2 = 272      # [128, 17]: cols 0-1 = b1 chunks, 2-16 = b2 (o*5+m)
OFF_M0 = 289        # w2 m=0 block: 6 sub-blocks (o*2+hc) x [128, 128]
COLS_SM0 = 289 + 768
# blob_band: band1 [128, 661] + band2 [128, 724]
OFF_BAND1 = 0
OFF_BAND2 = 661
COLS_BAND = 1385
# blob_b12: w2 m=1,2 blocks; blob_b34: m=3,4
COLS_B12 = 768 * 2
COLS_B34 = 768 + 384

N_WARMUP_MM = 5

USE_F32R = os.environ.get("METASR_F32R", "1") == "1"

_CACHE = {}


def _build_program(use_f32r):
    """Build + compile the single-core Bass program (same for all cores)."""
    nc = bacc.Bacc("TRN2", target_bir_lowering=False, debug=False)

    band_dt = F32R if use_f32r else F32
    w2_dt = F32R if use_f32r else F32
    blob_sm0_d = nc.dram_tensor(
        "blob_sm0", [128, COLS_SM0], w2_dt, kind="ExternalInput"
    )
    blob_band_d = nc.dram_tensor(
        "blob_band", [128, COLS_BAND], band_dt, kind="ExternalInput"
    )
    blob_b12_d = nc.dram_tensor(
        "blob_b12", [128, COLS_B12], w2_dt, kind="ExternalInput"
    )
    blob_b34_d = nc.dram_tensor(
        "blob_b34", [128, COLS_B34], w2_dt, kind="ExternalInput"
    )
    out48 = nc.dram_tensor("out48", [48, NPOS], F32, kind="ExternalOutput")

    with tile.TileContext(nc) as tc:
        with (
            tc.tile_pool(name="blobs", bufs=1) as blobs,
            tc.tile_pool(name="work", bufs=1) as work,
            tc.tile_pool(name="wpool", bufs=5) as wpool,
            tc.tile_pool(name="opool", bufs=1) as opool,
            tc.tile_pool(name="ps_small", bufs=2, space="PSUM") as ps_small,
            tc.tile_pool(name="ps_w", bufs=5, space="PSUM") as ps_w,
            tc.tile_pool(name="ps_rgb", bufs=1, space="PSUM") as ps_rgb,
        ):
            # 4 DMAs, 2 per HWDGE ring (ACT: sm0, b34 | SP: b12, band)
            blob_sm0 = blobs.tile([128, COLS_SM0], w2_dt, tag="blob_sm0")
            nc.scalar.dma_start(blob_sm0[:, :], blob_sm0_d[:, :])
            blob_b12 = blobs.tile([128, COLS_B12], w2_dt, tag="blob_b12")
            nc.sync.dma_start(blob_b12[:, :], blob_b12_d[:, :])
            blob_b34 = blobs.tile([128, COLS_B34], w2_dt, tag="blob_b34")
            nc.scalar.dma_start(blob_b34[:, :], blob_b34_d[:, :])
            blob_band = blobs.tile([128, COLS_BAND], band_dt, tag="blob_band")
            nc.sync.dma_start(blob_band[:, :], blob_band_d[:, :])

            sm0_f32 = blob_sm0.bitcast(F32) if use_f32r else blob_sm0
            w1_sb = sm0_f32[0:3, OFF_W1:OFF_W1 + 256]
            mlp_sb = sm0_f32[0:3, OFF_MLP:OFF_MLP + 16]
            b1b2 = sm0_f32[:, OFF_B1B2:OFF_B1B2 + 17]
            band_tiles = [
                blob_band[:, OFF_BAND1:OFF_BAND1 + 661],
                blob_band[:, OFF_BAND2:OFF_BAND2 + 724],
            ]

            def w2_slice(m, o, hc, msize):
                if m == 0:
                    base = OFF_M0 + (o * 2 + hc) * 128
                    return blob_sm0[:, base:base + msize]
                if m <= 2:
                    base = (m - 1) * 768 + (o * 2 + hc) * msize
                    return blob_b12[:, base:base + msize]
                base = (m - 3) * 768 + (o * 2 + hc) * msize
                return blob_b34[:, base:base + msize]

            # ---- PE warm-up: dummy zero matmuls into rgb_ps while DMAs run.
            # conv chunk 0 below uses start=True, which resets the PSUM
            # accumulation, so these contribute nothing to the result.
            rgb_ps = ps_rgb.tile([48, NPOS], F32, tag="rgb")
            warm = work.tile([128, 512], F32, tag="warm")
            nc.vector.memset(warm[:, :], 0.0)
            warm_bf = warm.bitcast(BF16)
            for _ in range(N_WARMUP_MM):
                nc.tensor.matmul(
                    rgb_ps[:, :], warm_bf[:, 0:48], warm_bf[:, 0:NPOS],
                    start=True, stop=True,
                )

            # ---- MLP layer 1: h_actT [256, 16] in two 128-chunks ----
            h_dt = F32R if use_f32r else F32
            h_sb = work.tile([128, 32], h_dt, tag="hact")
            for hc in range(2):
                ph = ps_small.tile([128, 16], F32, tag="ph")
                nc.tensor.matmul(
                    ph[:, :], w1_sb[:, hc * 128:(hc + 1) * 128], mlp_sb[:, :],
                    start=True, stop=True,
                )
                # relu(x + b1) = max(x + b1, 0) in one DVE op
                nc.vector.tensor_scalar(
                    h_sb[:, hc * 16:(hc + 1) * 16], ph[:, :],
                    b1b2[:, hc:hc + 1], 0.0,
                    mybir.AluOpType.add, mybir.AluOpType.max,
                )

            # ---- per K-chunk: W assembly (MLP layer 2) + conv matmul ----
            w_dt = F32R if use_f32r else F32
            for m, (bidx, roff, K) in enumerate(CHUNK_SPECS):
                msize = K
                w_sb = wpool.tile([128, 48], w_dt, tag="W")
                for o in range(3):
                    pw = ps_w.tile([128, 16], F32, tag="pw")
                    for hc in range(2):
                        nc.tensor.matmul(
                            pw[:msize, :],
                            w2_slice(m, o, hc, msize),
                            h_sb[:, hc * 16:(hc + 1) * 16],
                            start=(hc == 0), stop=(hc == 1),
                        )
                    nc.vector.tensor_scalar_add(
                        w_sb[:msize, o * 16:(o + 1) * 16], pw[:msize, :],
                        b1b2[:msize, 2 + o * 5 + m:3 + o * 5 + m],
                    )
                bt = band_tiles[bidx]
                rhs = bt[0:K, roff:roff + 8 * 66].rearrange(
                    "p (r c) -> p r c", c=66
                )[:, :, 0:64]
                nc.tensor.matmul(
                    rgb_ps[:, :], w_sb[:msize, :], rhs,
                    start=(m == 0), stop=(m == len(CHUNK_SPECS) - 1),
                )

            # ---- write out ----
            out_sb = opool.tile([48, NPOS], F32, tag="out")
            nc.vector.tensor_copy(out_sb[:, :], rgb_ps[:, :])
            nc.sync.dma_start(out48[:, :], out_sb[:, :])

    nc.compile()
    return nc


def _round_f32r(x):
    """Round fp32 to the fp32r-representable set (bf16 hi + bf16 lo pair)."""
    import ml_dtypes
    hi = x.astype(ml_dtypes.bfloat16).astype(np.float32)
    lo = (x - hi).astype(ml_dtypes.bfloat16).astype(np.float32)
    return hi + lo


def _host_prep(feat, w1, b1, w2, b2, use_f32r):
    """Pack shared blobs + per-core band blobs."""
    feat = np.ascontiguousarray(np.asarray(feat, dtype=np.float32))[0]  # [64,64,64]
    w1 = np.asarray(w1, dtype=np.float32)
    b1 = np.asarray(b1, dtype=np.float32)
    w2 = np.asarray(w2, dtype=np.float32)
    b2 = np.asarray(b2, dtype=np.float32)

    dydx = np.arange(16)
    mlpin = np.stack(
        [dydx // 4 / 4.0, dydx % 4 / 4.0, np.full(16, 0.25)], axis=0
    ).astype(np.float32)  # [3, 16]

    # tap-major permutations of w2/b2
    w2r = w2.reshape(256, 64, 9, 3)  # [h, c, t, o]
    w2p = np.empty((3, 256, 576), dtype=np.float32)
    b2r = b2.reshape(64, 9, 3)       # [c, t, o]
    b2p = np.empty((3, 576), dtype=np.float32)
    for blk, t in enumerate(TAP_ORDER):
        w2p[:, :, blk * 64:(blk + 1) * 64] = w2r[:, :, t, :].transpose(2, 0, 1)
        b2p[:, blk * 64:(blk + 1) * 64] = b2r[:, t, :].T

    if use_f32r:
        w2p = _round_f32r(w2p)

    blob_sm0 = np.zeros((128, COLS_SM0), dtype=np.float32)
    blob_sm0[0:3, OFF_W1:OFF_W1 + 256] = w1
    blob_sm0[0:3, OFF_MLP:OFF_MLP + 16] = mlpin
    blob_sm0[:, OFF_B1B2 + 0] = b1[0:128]
    blob_sm0[:, OFF_B1B2 + 1] = b1[128:256]
    for o in range(3):
        for m in range(5):
            msize = 128 if m < 4 else 64
            blob_sm0[:msize, OFF_B1B2 + 2 + o * 5 + m] = \
                b2p[o, 128 * m:128 * m + msize]
    for o in range(3):
        for hc in range(2):
            base = OFF_M0 + (o * 2 + hc) * 128
            blob_sm0[:, base:base + 128] = w2p[o, hc * 128:(hc + 1) * 128, 0:128]

    blob_b12 = np.empty((128, COLS_B12), dtype=np.float32)
    blob_b34 = np.empty((128, COLS_B34), dtype=np.float32)
    for m in range(1, 5):
        msize = 128 if m < 4 else 64
        dst = blob_b12 if m <= 2 else blob_b34
        moff = (m - 1) * 768 if m <= 2 else (m - 3) * 768
        for o in range(3):
            for hc in range(2):
                base = moff + (o * 2 + hc) * msize
                dst[:, base:base + msize] = \
                    w2p[o, hc * 128:(hc + 1) * 128, 128 * m:128 * m + msize]

    featp = np.zeros((64, 66, 66), dtype=np.float32)
    featp[:, 1:65, 1:65] = feat
    if use_f32r:
        featp = _round_f32r(featp)

    blobs_band = []
    for core in range(N_CORES):
        r0 = core * ROWS_PER_CORE
        band = featp[:, r0:r0 + BAND_ROWS, :].reshape(64, BAND_ROWS * 66)
        bb = np.zeros((128, COLS_BAND), dtype=np.float32)
        bb[0:64, OFF_BAND1 + 1:OFF_BAND1 + 661] = band
        bb[64:128, OFF_BAND1 + 0:OFF_BAND1 + 660] = band
        bb[0:64, OFF_BAND2 + 0:OFF_BAND2 + 660] = band
        bb[64:128, OFF_BAND2 + 64:OFF_BAND2 + 724] = band
        blobs_band.append(bb)
    return blob_sm0, blob_b12, blob_b34, blobs_band


def _assemble(per_core_out48):
    """[8 x [48, 512]] -> [1, 3, 256, 256]."""
    full = np.stack(per_core_out48)                      # [core, 48, 512]
    full = full.reshape(8, 3, 4, 4, 8, 64)               # [core, o, dy, dx, r, x]
    rgb = full.transpose(1, 0, 4, 2, 5, 3).reshape(3, 256, 256)
    return np.ascontiguousarray(rgb)[None]


def get_program():
    key = ("nc", USE_F32R)
    if key not in _CACHE:
        _CACHE[key] = _build_program(USE_F32R)
    return _CACHE[key]


def run(feat, w1, b1, w2, b2, out_h, out_w, trace=False, **spmd_kwargs):
    assert int(out_h) == 256 and int(out_w) == 256
    nc = get_program()
    blob_sm0, blob_b12, blob_b34, blobs_band = _host_prep(
        feat, w1, b1, w2, b2, USE_F32R
    )
    in_maps = [
        {"blob_sm0": blob_sm0, "blob_b12": blob_b12, "blob_b34": blob_b34,
         "blob_band": blobs_band[core]}
        for core in range(N_CORES)
    ]
    res = run_bass_kernel_spmd(
        nc, in_maps, core_ids=list(range(N_CORES)), trace=trace, **spmd_kwargs
    )
    out = _assemble([res.results[core]["out48"] for core in range(N_CORES)])
    return out, res


def kernel(feat, w1, b1, w2, b2, out_h, out_w):
    out, _ = run(feat, w1, b1, w2, b2, out_h, out_w, trace=False)
    return out



# revision 2
# speedup vs baseline: 1.2686x; 1.2686x over previous
"""MetaSR super-resolution Trainium2 kernel (bf16 edition).

Structure exploited: out_h=out_w=256 with H=W=64 LR grid means the scale
factor is exactly 4, so the nearest-neighbor gather index is iy=oy//4,
ix=ox//4 and the per-query MLP input collapses to 16 distinct subpixel
phases [dy/4, dx/4, 0.25].  The whole model becomes:

  1. h    = relu(mlp_in @ w1 + b1)              [16, 256]
  2. predw = h @ w2 + b2                        [16, 576, 3]
  3. rgb[o, 4*iy+dy, 4*ix+dx] =
       sum_{c,ki,kj} feat[c, iy+ki-1, ix+kj-1] * predw[(dy,dx), c*9+ki*3+kj, o]
     i.e. a 3x3 conv with 64 in / 48 out channels + pixel shuffle.

Sharding: data-parallel over LR rows (8 rows per core, 10-row halo band),
weights replicated; steps 1+2 are recomputed on every core (tiny).

The conv contraction (K = 9 taps x 64 ch = 576) is chunked K=128 by pairing
taps.  Each core holds the zero-padded band twice in a 128-partition tile at
free-dim offsets that differ by the two taps' shift delta, so one K=128
matmul consumes two taps without materializing the unfolded tensor:
  band free index = r*66 + x  (66-wide zero-padded rows), tap (ki,kj) shift
  = ki*66+kj; taps are paired with shift deltas 1 or 64.

All large operands are bf16 (host-side cast): w2 chunks, the band, h, the
conv stationary W and the output (upcast to f32 host-side).  Verified
end-to-end rel err ~4e-3 vs the f32 reference (budget 2e-2).

DMA order on the Sync queue follows consumption order: tiny w1+mlpin blob
first (unblocks the MLP), then bias+w2-m0, then m1+m2, then m3+m4; the band
rides the Scalar queue.  A run of dummy matmuls (zero scratch data) keeps
the PE continuously busy during the DMA phase so its clock ramps to the
full 2.4 GHz p-state before the real matmuls run.
"""

import os

import numpy as np

try:
    import concourse.bass as bass
except ImportError:  # fall back to the repo checkout
    import sys
    sys.path.insert(0, "/opt/trn_rl_repo")
    import concourse.bass as bass
import concourse.mybir as mybir
import concourse.tile as tile
from concourse import bacc
from concourse.bass_utils import run_bass_kernel_spmd

F32 = mybir.dt.float32
BF16 = mybir.dt.bfloat16
N_CORES = 8
ROWS_PER_CORE = 8          # LR rows per core
BAND_ROWS = ROWS_PER_CORE + 2
NPOS = ROWS_PER_CORE * 64  # 512 LR positions per core

# Tap order for K-chunking.  Taps t = ki*3+kj have band shift ki*66+kj:
#   t:      0   1   2   3    4    5    6    7    8
#   shift:  0   1   2   66   67   68   132  133  134
# chunk0: [t0; t1] band1 off 1 | chunk1: [t3; t2] band2 off 66
# chunk2: [t4; t5] band1 off 68 | chunk3: [t6; t7] band1 off 133
# chunk4: [t8] band2 off 134 (K=64)
TAP_ORDER = [0, 1, 3, 2, 4, 5, 6, 7, 8]
CHUNK_SPECS = [  # (band_tile_idx, rhs_offset, K)
    (0, 1, 128),
    (1, 66, 128),
    (0, 68, 128),
    (0, 133, 128),
    (1, 134, 64),
]

# blob_w1 [3, 272] bf16: w1 [3,256] | mlpin [3,16]
COLS_W1 = 272
# blob_a [128, 802] bf16: bias (17 f32 = 34 bf16 cols) | w2 m0 (6 x 128)
OFF_M0 = 34
COLS_A = 34 + 768
# blob_b: w2 m=1,2 | blob_c: m=3 (768) + m=4 (6 x 64)
COLS_B = 768 * 2
COLS_C = 768 + 384
# blob_band: band1 [128, 661] + band2 [128, 724]
OFF_BAND2 = 661
COLS_BAND = 1385

WARM_BIG = int(os.environ.get("METASR_WARM_BIG", "4"))
WARM_SMALL = int(os.environ.get("METASR_WARM_SMALL", "8"))

_CACHE = {}


def _build_program(warm_big, warm_small):
    """Build + compile the single-core Bass program (same for all cores)."""
    nc = bacc.Bacc("TRN2", target_bir_lowering=False, debug=False)

    blob_w1_d = nc.dram_tensor("blob_w1", [3, COLS_W1], BF16, kind="ExternalInput")
    blob_a_d = nc.dram_tensor("blob_a", [128, COLS_A], BF16, kind="ExternalInput")
    blob_b_d = nc.dram_tensor("blob_b", [128, COLS_B], BF16, kind="ExternalInput")
    blob_c_d = nc.dram_tensor("blob_c", [128, COLS_C], BF16, kind="ExternalInput")
    blob_band_d = nc.dram_tensor(
        "blob_band", [128, COLS_BAND], BF16, kind="ExternalInput"
    )
    out48 = nc.dram_tensor("out48", [48, NPOS], BF16, kind="ExternalOutput")

    with tile.TileContext(nc) as tc:
        with (
            tc.tile_pool(name="blobs", bufs=1) as blobs,
            tc.tile_pool(name="work", bufs=1) as work,
            tc.tile_pool(name="wpool", bufs=5) as wpool,
            tc.tile_pool(name="opool", bufs=1) as opool,
            tc.tile_pool(name="ps_small", bufs=2, space="PSUM") as ps_small,
            tc.tile_pool(name="ps_w", bufs=5, space="PSUM") as ps_w,
            tc.tile_pool(name="ps_rgb", bufs=1, space="PSUM") as ps_rgb,
        ):
            # DMAs in consumption order: Sync gets the w2 stream, Scalar
            # the band (needed last).
            blob_w1 = blobs.tile([3, COLS_W1], BF16, tag="blob_w1")
            nc.sync.dma_start(blob_w1[:, :], blob_w1_d[:, :])
            blob_a = blobs.tile([128, COLS_A], BF16, tag="blob_a")
            nc.sync.dma_start(blob_a[:, :], blob_a_d[:, :])
            blob_b = blobs.tile([128, COLS_B], BF16, tag="blob_b")
            nc.sync.dma_start(blob_b[:, :], blob_b_d[:, :])
            blob_c = blobs.tile([128, COLS_C], BF16, tag="blob_c")
            nc.sync.dma_start(blob_c[:, :], blob_c_d[:, :])
            blob_band = blobs.tile([128, COLS_BAND], BF16, tag="blob_band")
            nc.scalar.dma_start(blob_band[:, :], blob_band_d[:, :])

            w1_sb = blob_w1[0:3, 0:256]
            mlp_sb = blob_w1[0:3, 256:272]
            bias = blob_a.bitcast(F32)[:, 0:17]
            band_tiles = [
                blob_band[:, 0:661],
                blob_band[:, OFF_BAND2:OFF_BAND2 + 724],
            ]

            def w2_slice(m, o, hc, msize):
                if m == 0:
                    base = OFF_M0 + (o * 2 + hc) * 128
                    return blob_a[:, base:base + msize]
                if m <= 2:
                    base = (m - 1) * 768 + (o * 2 + hc) * msize
                    return blob_b[:, base:base + msize]
                base = (m - 3) * 768 + (o * 2 + hc) * msize
                return blob_c[:, base:base + msize]

            # ---- PE warm-up: dummy zero matmuls into rgb_ps while DMAs run.
            # conv chunk 0 below uses start=True, which resets the PSUM
            # accumulation, so these contribute nothing to the result.  Mix
            # of long + short matmuls keeps the PE clock ramping without
            # delaying the first real matmul by more than ~one short one.
            rgb_ps = ps_rgb.tile([48, NPOS], F32, tag="rgb")
            warm = work.tile([128, 512], BF16, tag="warm")
            nc.vector.memset(warm[:, :], 0.0)
            for _ in range(warm_big):
                nc.tensor.matmul(
                    rgb_ps[:, :], warm[:, 0:48], warm[:, 0:NPOS],
                    start=True, stop=True,
                )
            for _ in range(warm_small):
                nc.tensor.matmul(
                    rgb_ps[:, 0:128], warm[:, 0:48], warm[:, 0:128],
                    start=True, stop=True,
                )

            # ---- MLP layer 1: h_actT [256, 16] in two 128-chunks ----
            h_sb = work.tile([128, 32], BF16, tag="hact")
            for hc in range(2):
                ph = ps_small.tile([128, 16], F32, tag="ph")
                nc.tensor.matmul(
                    ph[:, :], w1_sb[:, hc * 128:(hc + 1) * 128], mlp_sb[:, :],
                    start=True, stop=True,
                )
                # relu(x + b1) = max(x + b1, 0) in one DVE op
                nc.vector.tensor_scalar(
                    h_sb[:, hc * 16:(hc + 1) * 16], ph[:, :],
                    bias[:, hc:hc + 1], 0.0,
                    mybir.AluOpType.add, mybir.AluOpType.max,
                )

            # ---- per K-chunk: W assembly (MLP layer 2) + conv matmul ----
            for m, (bidx, roff, K) in enumerate(CHUNK_SPECS):
                msize = K
                w_sb = wpool.tile([128, 48], BF16, tag="W")
                for o in range(3):
                    pw = ps_w.tile([128, 16], F32, tag="pw")
                    for hc in range(2):
                        nc.tensor.matmul(
                            pw[:msize, :],
                            w2_slice(m, o, hc, msize),
                            h_sb[:, hc * 16:(hc + 1) * 16],
                            start=(hc == 0), stop=(hc == 1),
                        )
                    nc.vector.tensor_scalar_add(
                        w_sb[:msize, o * 16:(o + 1) * 16], pw[:msize, :],
                        bias[:msize, 2 + o * 5 + m:3 + o * 5 + m],
                    )
                bt = band_tiles[bidx]
                rhs = bt[0:K, roff:roff + 8 * 66].rearrange(
                    "p (r c) -> p r c", c=66
                )[:, :, 0:64]
                nc.tensor.matmul(
                    rgb_ps[:, :], w_sb[:msize, :], rhs,
                    start=(m == 0), stop=(m == len(CHUNK_SPECS) - 1),
                )

            # ---- write out (bf16, host upcasts) ----
            out_sb = opool.tile([48, NPOS], BF16, tag="out")
            nc.vector.tensor_copy(out_sb[:, :], rgb_ps[:, :])
            nc.sync.dma_start(out48[:, :], out_sb[:, :])

    nc.compile()
    return nc


def _host_prep(feat, w1, b1, w2, b2):
    """Pack shared blobs + per-core band blobs (bf16)."""
    import ml_dtypes
    bf16 = ml_dtypes.bfloat16
    feat = np.ascontiguousarray(np.asarray(feat, dtype=np.float32))[0]  # [64,64,64]
    w1 = np.asarray(w1, dtype=np.float32)
    b1 = np.asarray(b1, dtype=np.float32)
    w2 = np.asarray(w2, dtype=np.float32)
    b2 = np.asarray(b2, dtype=np.float32)

    dydx = np.arange(16)
    mlpin = np.stack(
        [dydx // 4 / 4.0, dydx % 4 / 4.0, np.full(16, 0.25)], axis=0
    ).astype(np.float32)  # [3, 16]

    # tap-major permutations of w2/b2
    w2r = w2.reshape(256, 64, 9, 3)  # [h, c, t, o]
    w2p = np.empty((3, 256, 576), dtype=np.float32)
    b2r = b2.reshape(64, 9, 3)       # [c, t, o]
    b2p = np.empty((3, 576), dtype=np.float32)
    for blk, t in enumerate(TAP_ORDER):
        w2p[:, :, blk * 64:(blk + 1) * 64] = w2r[:, :, t, :].transpose(2, 0, 1)
        b2p[:, blk * 64:(blk + 1) * 64] = b2r[:, t, :].T
    w2p = w2p.astype(bf16)

    blob_w1 = np.zeros((3, COLS_W1), dtype=bf16)
    blob_w1[:, 0:256] = w1.astype(bf16)
    blob_w1[:, 256:272] = mlpin.astype(bf16)

    bias = np.zeros((128, 17), dtype=np.float32)
    bias[:, 0] = b1[0:128]
    bias[:, 1] = b1[128:256]
    for o in range(3):
        for m in range(5):
            msize = 128 if m < 4 else 64
            bias[:msize, 2 + o * 5 + m] = b2p[o, 128 * m:128 * m + msize]

    blob_a = np.zeros((128, COLS_A), dtype=bf16)
    blob_a[:, 0:34] = bias.view(np.uint32).view(np.uint16).view(bf16)
    for o in range(3):
        for hc in range(2):
            base = OFF_M0 + (o * 2 + hc) * 128
            blob_a[:, base:base + 128] = w2p[o, hc * 128:(hc + 1) * 128, 0:128]

    blob_b = np.empty((128, COLS_B), dtype=bf16)
    blob_c = np.empty((128, COLS_C), dtype=bf16)
    for m in range(1, 5):
        msize = 128 if m < 4 else 64
        dst = blob_b if m <= 2 else blob_c
        moff = (m - 1) * 768 if m <= 2 else (m - 3) * 768
        for o in range(3):
            for hc in range(2):
                base = moff + (o * 2 + hc) * msize
                dst[:, base:base + msize] = \
                    w2p[o, hc * 128:(hc + 1) * 128, 128 * m:128 * m + msize]

    featp = np.zeros((64, 66, 66), dtype=bf16)
    featp[:, 1:65, 1:65] = feat.astype(bf16)

    blobs_band = []
    for core in range(N_CORES):
        r0 = core * ROWS_PER_CORE
        band = featp[:, r0:r0 + BAND_ROWS, :].reshape(64, BAND_ROWS * 66)
        bb = np.zeros((128, COLS_BAND), dtype=bf16)
        bb[0:64, 1:661] = band
        bb[64:128, 0:660] = band
        bb[0:64, OFF_BAND2:OFF_BAND2 + 660] = band
        bb[64:128, OFF_BAND2 + 64:OFF_BAND2 + 724] = band
        blobs_band.append(bb)
    return blob_w1, blob_a, blob_b, blob_c, blobs_band


def _assemble(per_core_out48):
    """[8 x [48, 512] bf16] -> [1, 3, 256, 256] f32."""
    full = np.stack([np.asarray(o).astype(np.float32) for o in per_core_out48])
    full = full.reshape(8, 3, 4, 4, 8, 64)               # [core, o, dy, dx, r, x]
    rgb = full.transpose(1, 0, 4, 2, 5, 3).reshape(3, 256, 256)
    return np.ascontiguousarray(rgb)[None]


def get_program():
    key = ("nc", WARM_BIG, WARM_SMALL)
    if key not in _CACHE:
        _CACHE[key] = _build_program(WARM_BIG, WARM_SMALL)
    return _CACHE[key]


def run(feat, w1, b1, w2, b2, out_h, out_w, trace=False, **spmd_kwargs):
    assert int(out_h) == 256 and int(out_w) == 256
    nc = get_program()
    blob_w1, blob_a, blob_b, blob_c, blobs_band = _host_prep(feat, w1, b1, w2, b2)
    in_maps = [
        {"blob_w1": blob_w1, "blob_a": blob_a, "blob_b": blob_b,
         "blob_c": blob_c, "blob_band": blobs_band[core]}
        for core in range(N_CORES)
    ]
    res = run_bass_kernel_spmd(
        nc, in_maps, core_ids=list(range(N_CORES)), trace=trace, **spmd_kwargs
    )
    out = _assemble([res.results[core]["out48"] for core in range(N_CORES)])
    return out, res


def kernel(feat, w1, b1, w2, b2, out_h, out_w):
    out, _ = run(feat, w1, b1, w2, b2, out_h, out_w, trace=False)
    return out
